# revision 18
# baseline (speedup 1.0000x reference)
"""DCT block extractor kernel for 8 TRN2 NeuronCores (pure data parallel).

Math: for each 8x8 block of each [512,512] image, the 2D-DFT bin (u,v) is
  X[u,v] = sum_{r,s} x[r,s] * exp(-2*pi*i*(u*r + v*s)/8)
We need |X| at 6 (u,v) bands, averaged over all 64x64 blocks.

Implementation: contraction over the in-block row index r is done on the
TensorEngine partition axis (block-diagonal weights over 8 row-groups per
64-row chunk); contraction over the in-block column index s is done by PSUM
accumulation across 8 matmuls, each reading a stride-8 column slice of the
image rows. One matmul per (chunk, s):
  lhsT = W[s]  [64, 128]  (k = gi*8+r; Re at m=band*8+gi, Im at m=64+band*8+gi)
  rhs  = rows[:, s::8]    [64, 512]   (free = (img in batch, gj))
Inputs are cast fp32->fp16 by the (gpsimd software-DGE) DMA so the matmul
runs single-pass at 1 cycle/row with fast weight load; PSUM accumulates fp32.

The pipeline is tile-granular: all input DMAs are issued up-front (the whole
25 MB per-core input fits resident in SBUF, so the HBM stream never stalls on
buffer reuse), and each tile (2 chunks of 64 rows; the last 4 tiles are
1 chunk to shorten the drain) flows DMA -> deinterleave (DVE, lookahead 1
tile) -> matmuls (s-outer, chunk-inner: weights switch every 2 matmuls and
PSUM banks alternate, keeping the PE near its 236ns/matmul rate) -> per-chunk
Square/Sqrt (ACT, fp16 out) -> add+reduce (DVE) into a persistent [48, 24]
accumulator, with a single tiny output DMA at the very end.  This keeps every
engine within ~1 tile of the DMA stream (the roofline resource) instead of
batch-serializing on PSUM drain.
Final tiny mean/reshape is done on host from a [48, 24] per-core result.
"""

import os
import sys

import numpy as np

for _p in ("/opt/trn_rl_repo",):
    if os.path.isdir(_p) and _p not in sys.path:
        sys.path.insert(0, _p)

import concourse.bass as bass  # noqa: E402
import concourse.tile as tile  # noqa: E402
from concourse import bacc, mybir  # noqa: E402
from concourse.bass_utils import run_bass_kernel_spmd  # noqa: E402

# Problem shape (hardcoded per contract)
B, C, H, W = 64, 3, 512, 512
N_CORES = 8
BL = B // N_CORES   # 8 batch rows per core
NIMG = BL * C       # 24 images per core (flattened (b, c))
IPB = 8             # images per device-batch
NBATCH = NIMG // IPB  # 3 device-batches
NCHUNK = 8          # 64-row chunks per image
GJ = 64             # block-columns
NFREE = IPB * GJ    # 512 matmul free size
NBANDS = 6
# Input tiles as (first global chunk, n chunks): 2-chunk tiles for the bulk,
# 1-chunk tiles at the end so the pipeline drain after the last HBM byte is
# one short chunk chain instead of a 2-chunk one.
TILES = [(2 * t, 2) for t in range(10)] + [(20 + k, 1) for k in range(4)]

FREQ_BANDS = np.array([[0, 1], [1, 0], [1, 1], [2, 2], [3, 3], [4, 4]]) % 8

BENCH = False          # set True (e.g. from test.py) to profile
BENCH_KWARGS = {}
LAST_EXEC_NS = None
LAST_RESULTS = None

_CACHED_NC = None


def _weights() -> np.ndarray:
    """W[s] in [8, 128, 128] fp16: Re at m=band*8+gi, Im at m=64+band*8+gi.

    Rows 64:128 duplicate rows 0:64 so lhsT can be sliced at base partition
    0 or 64 to match the rhs chunk's base partition."""
    w = np.zeros((8, 64, 128), dtype=np.float32)
    r = np.arange(8)
    for s in range(8):
        for b, (u, v) in enumerate(FREQ_BANDS):
            th = 2.0 * np.pi * (u * r + v * s) / 8.0
            cs, sn = np.cos(th), np.sin(th)
            for gi in range(8):
                w[s, gi * 8 : gi * 8 + 8, b * 8 + gi] = cs
                w[s, gi * 8 : gi * 8 + 8, 64 + b * 8 + gi] = sn
    w = np.concatenate([w, w], axis=1).astype(np.float16)
    # pre-transposed to [k, s, m] so the weights DMA is 128 contiguous 2KB
    # rows instead of ~1k 256B descriptors (which would eat ~5us of DMA
    # queue time ahead of the first input tile)
    return np.ascontiguousarray(w.transpose(1, 0, 2))


def _build():
    nc = bacc.Bacc("TRN2", target_bir_lowering=False, debug=False, num_devices=N_CORES)
    f32 = mybir.dt.float32
    f16 = mybir.dt.float16

    x_d = nc.dram_tensor("x", [NIMG, H, W], f32, kind="ExternalInput")
    w_d = nc.dram_tensor("w", [128, 8, 128], f16, kind="ExternalInput")
    out_d = nc.dram_tensor("out", [48, NIMG], f32, kind="ExternalOutput")

    with tile.TileContext(nc) as tc:
        with (
            tc.tile_pool(name="consts", bufs=1) as consts,
            tc.tile_pool(name="inp", bufs=len(TILES)) as inp,
            tc.tile_pool(name="deint", bufs=6) as deint,
            tc.tile_pool(name="psum", bufs=1, space="PSUM") as psum_pool,
            tc.tile_pool(name="work", bufs=6) as work,
            tc.tile_pool(name="redp", bufs=4) as redp,
        ):
            w_sb = consts.tile([128, 8, 128], f16)
            nc.sync.dma_start(out=w_sb, in_=w_d[:])

            # All input DMAs up-front: the tiles cover the whole per-core
            # input, so the 16 DMA queues stream HBM back-to-back.  One DMA
            # per tile (64/128 consecutive image rows -> partitions); the
            # software-DGE DMA casts fp32 -> fp16 in flight.
            in_tiles = []
            for c0, nch in TILES:
                bt, cb = divmod(c0, NCHUNK)
                it = inp.tile([64 * nch, IPB, W], f16, name="it")
                nc.gpsimd.dma_start(
                    out=it,
                    in_=x_d[
                        bt * IPB : (bt + 1) * IPB,
                        cb * 64 : (cb + nch) * 64,
                        :,
                    ].transpose([1, 0, 2]),
                )
                in_tiles.append(it)

            # PE warm-up: ~24 dense dummy matmuls (~7us of PE activity) to
            # trip the HAM clock gate to 8/8 (2.4 GHz) before the real work.
            # Bank 7 is reused by chunk 7 only much later, so no conflict.
            warm = consts.tile([128, 512], f16)
            nc.vector.memset(warm, 0.0)
            ps_w = psum_pool.tile([128, 512], f32, tag="ps7", name="ps_w")
            for i in range(24):
                nc.tensor.matmul(ps_w, warm[:, 0:128], warm, start=(i == 0), stop=(i == 23))

            out_acc = consts.tile([48, NIMG], f32)

            # pair-deinterleave columns: col gj*8+s -> s_hi*256 + gj*4 + s_lo
            # (s = 4*s_hi + s_lo) so matmul rhs reads at stride 4 (8 bytes),
            # below the 16-byte SBUF line-crossing cliff. Reads here are
            # 4-contiguous-fp16 runs (8B) -> also below the cliff. DVE does
            # these 8B-granular strided copies at 2-4x mode (~0.7us each).
            def emit_deint(jt):
                c0, nch = TILES[jt]
                it = in_tiles[jt]
                dt_ = deint.tile([64 * nch, IPB, 2, 256], f16, name="dt")
                it_v = it.rearrange("p i (g e) -> p i g e", e=8)
                for s_hi in range(2):
                    nc.vector.tensor_copy(
                        dt_[:, :, s_hi].rearrange("p i (g q) -> p i g q", q=4),
                        it_v[:, :, :, 4 * s_hi : 4 * s_hi + 4],
                    )
                return dt_

            # lookahead 2: tile jt's mag work sits behind deint(jt+2) in
            # the DVE queue, so the deint -> matmul -> ACT -> mag -> next
            # deint dependency loop spans 3 tiles (~8us of latency per
            # 3*5.6us of DMA) and the PE is fed just-in-time at DMA pace.
            # The four 1-chunk drain tiles (10-13) are emitted two per
            # iteration at jt=8,9 so their deints run the moment their DMA
            # lands and the drain tracks the tail of the HBM stream.
            DEINT_AT = {jt: [jt + 2] for jt in range(8)}
            DEINT_AT[8] = [10, 11]
            DEINT_AT[9] = [12, 13]
            deint_tiles = [emit_deint(0), emit_deint(1)]
            for jt, (c0, nch) in enumerate(TILES):
                for tgt in DEINT_AT.get(jt, []):
                    deint_tiles.append(emit_deint(tgt))
                dt_ = deint_tiles[jt]
                pss = [
                    psum_pool.tile(
                        [128, NFREE], f32, tag=f"ps{(c0 + k) % 8}", name="ps"
                    )
                    for k in range(nch)
                ]
                rhs_vs = [
                    dt_[64 * k : 64 * k + 64].rearrange(
                        "k i h (g q) -> k i h g q", q=4
                    )
                    for k in range(nch)
                ]
                for s in range(8):
                    for k in range(nch):
                        # lhsT must sit on the same partitions as the rhs
                        # slice; w rows 64:128 duplicate 0:64 for this.
                        base = rhs_vs[k].base_partition()
                        nc.tensor.matmul(
                            pss[k],
                            w_sb[base : base + 64, s, :],
                            rhs_vs[k][:, :, s // 4, :, s % 4],
                            start=(s == 0),
                            stop=(s == 7),
                        )
                # |X| = sqrt(re^2 + im^2); squares in fp16 (values < ~1100,
                # well inside fp16 range) halve the DVE add/reduce cost.
                # sq_re/sq_im are separate tiles: DVE TensorTensor requires
                # both inputs at the same base partition.
                for k in range(nch):
                    c = c0 + k
                    bt = c // NCHUNK
                    ps = pss[k]
                    sq_re = work.tile([48, NFREE], f16)
                    sq_im = work.tile([48, NFREE], f16)
                    nc.scalar.square(sq_re, ps[0:48])
                    nc.scalar.square(sq_im, ps[64:112])
                    ss = work.tile([48, NFREE], f16)
                    nc.vector.tensor_add(ss, sq_re, sq_im)
                    mag = work.tile([48, NFREE], f16)
                    nc.scalar.sqrt(mag, ss)
                    mag_v = mag.rearrange("p (i g) -> p i g", g=GJ)
                    ocol = out_acc[:, bt * IPB : (bt + 1) * IPB]
                    if c % NCHUNK == 0:
                        nc.vector.reduce_sum(
                            out=ocol, in_=mag_v, axis=mybir.AxisListType.X
                        )
                    else:
                        red = redp.tile([48, IPB], f32)
                        nc.vector.reduce_sum(
                            out=red, in_=mag_v, axis=mybir.AxisListType.X
                        )
                        nc.vector.tensor_add(ocol, ocol, red)

            nc.sync.dma_start(out=out_d[:], in_=out_acc)

    nc.compile()
    return nc


def kernel(x: np.ndarray) -> np.ndarray:
    global _CACHED_NC, LAST_EXEC_NS, LAST_RESULTS
    x = np.ascontiguousarray(np.asarray(x, dtype=np.float32))
    assert x.shape == (B, C, H, W), x.shape

    if _CACHED_NC is None:
        _CACHED_NC = _build()
    nc = _CACHED_NC

    w = _weights()
    in_maps = [
        {"x": x[i * BL : (i + 1) * BL].reshape(NIMG, H, W), "w": w}
        for i in range(N_CORES)
    ]
    kwargs = dict(BENCH_KWARGS)
    if BENCH:
        kwargs.setdefault("trace", True)
    res = run_bass_kernel_spmd(nc, in_maps, core_ids=list(range(N_CORES)), **kwargs)
    LAST_EXEC_NS = res.exec_time_ns
    LAST_RESULTS = res

    outs = []
    for i in range(N_CORES):
        o = np.asarray(res.results[i]["out"], dtype=np.float64)  # [48, 24]
        o = o.reshape(NBANDS, 8, NIMG)  # [band, gi_l, img]
        o = o.sum(axis=1) / 4096.0      # mean over all 64x64 blocks
        outs.append(o.T.reshape(BL, C * NBANDS))  # img = b_l*C + ch
    return np.concatenate(outs, axis=0).astype(np.float32)
